# revision 1
# baseline (speedup 1.0000x reference)
"""Causal self-attention (B=4, T=2048, C=1024, H=16, D=64) on 8 trn2 cores.

Sharding: core c handles batch b = c//2 and head-group hg = c%2 (8 heads).
qkv projection is column-parallel, attention is head-parallel, out_proj is
row-parallel; the final 2-way partial-sum + bias happens on host.

Per-core device program, pipelined over head PAIRS so the qkv projection of
pair p+1 overlaps the attention of pair p:
  per pair p (heads 2p, 2p+1, living on partition halves 0-63 / 64-127):
    - qkvT = (W_slice.T @ x.T) + bias -> qT,kT [feat, tok], vT [feat, tok]
    - v2 = PE-transpose(vT) with interleaved ones columns (the ones column
      makes the attn@v matmul also emit the softmax denominator row)
    - per q-chunk: scoresT = kT.T@qT (causal-skipped + sliced), exp on ACT,
      128-wide triangle mask on DVE, ctxT_ext = [v|1].T @ exp in PSUM;
      denominators collected by DMA, batched reciprocal, broadcast across
      partitions by a partition-step-0 DMA, normalize ctx in place
  tail: y_partial = ctx_stacked.T @ W_out_slice -> DRAM
"""

import os
import sys

for _p in ("/opt/trn_rl_repo", "/root/.axon_site/_ro/trn_rl_repo"):
    if os.path.isdir(_p) and _p not in sys.path:
        sys.path.insert(0, _p)

import numpy as np

B, T, C = 4, 2048, 1024
H, D = 16, 64
NCORES = 8
HPC = 8          # heads per core
FQ = HPC * D     # 512 per-core q (=k=v) feature count
TK = T // 128    # 16 token tiles of 128
V2W = 130        # v2 per-ktile width: 64 + 1 + 64 + 1

_CACHE = {}


def _build_program():
    import concourse.bacc as bacc
    import concourse.tile as tile
    import concourse.mybir as mybir
    from contextlib import ExitStack

    f32 = mybir.dt.float32
    f32r = mybir.dt.float32r
    AF = mybir.ActivationFunctionType

    nc = bacc.Bacc("TRN2", target_bir_lowering=False, debug=False)

    x_t = nc.dram_tensor("x_t", [C, T], f32r, kind="ExternalInput").ap()
    w_s = nc.dram_tensor("w_s", [C, 3 * FQ], f32r, kind="ExternalInput").ap()
    b_s = nc.dram_tensor("b_s", [3 * FQ], f32, kind="ExternalInput").ap()
    w_o = nc.dram_tensor("w_o", [FQ, C], f32r, kind="ExternalInput").ap()
    tri_d = nc.dram_tensor("tri", [128, 128], f32, kind="ExternalInput").ap()
    idm_d = nc.dram_tensor("idm", [128, 128], f32r, kind="ExternalInput").ap()
    onec_d = nc.dram_tensor("onec", [128, 64], f32r, kind="ExternalInput").ap()
    y_d = nc.dram_tensor("y", [T, C], f32, kind="ExternalOutput").ap()

    with tile.TileContext(nc) as tc, ExitStack() as ctx:
        # ---- whole-kernel persistents ----
        pp = ctx.enter_context(tc.tile_pool(name="persist", bufs=1))
        ident = pp.tile([128, 128], f32r, tag="ident", name="ident")
        nc.sync.dma_start(out=ident, in_=idm_d)
        b_sb = pp.tile([128, 12], f32, tag="bias", name="b_sb")
        nc.sync.dma_start(out=b_sb, in_=b_s.rearrange("(f p) -> p f", p=128))
        tri_sb = pp.tile([128, 128], f32, tag="tri", name="tri_sb")
        nc.sync.dma_start(out=tri_sb, in_=tri_d)

        ctx_pool = ctx.enter_context(tc.tile_pool(name="ctxp", bufs=1))
        ctx_sb = [ctx_pool.tile([128, T], f32r, tag=f"c{i}", name=f"ctx_sb{i}")
                  for i in range(4)]

        # psum pools that span the whole pair pipeline (8 banks total:
        # qkv 1 + transpose 1 + scores 4 + ctx 2)
        ps1 = ctx.enter_context(tc.tile_pool(name="ps1", bufs=1, space="PSUM"))
        pst = ctx.enter_context(tc.tile_pool(name="pst", bufs=1, space="PSUM"))

        with tc.tile_pool(name="xres", bufs=1) as xp, \
             tc.tile_pool(name="wstr", bufs=2) as wp, \
             tc.tile_pool(name="qkq", bufs=2) as qkq, \
             tc.tile_pool(name="vTq", bufs=2) as vTq, \
             tc.tile_pool(name="v2q", bufs=2) as v2q, \
             tc.tile_pool(name="collp", bufs=3) as coll_pool, \
             tc.tile_pool(name="bcq", bufs=3) as bcq, \
             tc.tile_pool(name="dscr", bufs=4, space="DRAM") as dscr, \
             tc.tile_pool(name="stg", bufs=4) as stg, \
             tc.tile_pool(name="esb", bufs=3) as esb, \
             tc.tile_pool(name="scps", bufs=2, space="PSUM") as scps, \
             tc.tile_pool(name="cxps", bufs=2, space="PSUM") as cxps:

            x_sb = [xp.tile([128, T], f32r, tag=f"x{ks}", name=f"x_sb{ks}")
                    for ks in range(8)]
            # chunked tcn-outer so the first token-chunk's 8 k-slices arrive
            # quickly and the first psum accumulation can start early
            for tcn in range(4):
                for ks in range(8):
                    nc.sync.dma_start(
                        out=x_sb[ks][:, tcn * 512:(tcn + 1) * 512],
                        in_=x_t[ks * 128:(ks + 1) * 128,
                                tcn * 512:(tcn + 1) * 512])

            for pair in range(4):
                # ---- qkv projection for this pair (v first, then q, k) ----
                qp = qkq.tile([128, T], f32r, tag="qp", name="q_p")
                kp = qkq.tile([128, T], f32r, tag="kp", name="k_p")
                vT = vTq.tile([128, T], f32r, tag="vT", name="vT_p")
                v2 = v2q.tile([128, TK * V2W], f32r, tag="v2", name="v2_p")
                v2v = v2.rearrange("p (t w) -> p t w", w=V2W)
                onec3 = onec_d[:, 0:16].rearrange("p (t o) -> p t o", o=1)
                nc.sync.dma_start(out=v2v[:, :, 64:65], in_=onec3)
                nc.sync.dma_start(out=v2v[:, :, 129:130], in_=onec3)

                for ft, dest in ((8 + pair, vT), (pair, qp), (4 + pair, kp)):
                    wt = [wp.tile([128, 128], f32r, tag=f"w{ks}",
                                  name=f"wt{ks}") for ks in range(8)]
                    for ks in range(8):
                        nc.gpsimd.dma_start(
                            out=wt[ks],
                            in_=w_s[ks * 128:(ks + 1) * 128,
                                    ft * 128:(ft + 1) * 128])
                    for tcn in range(4):
                        # double-buffer the qkv psum by ping-ponging between
                        # the two single-slot pools; the transpose slot is
                        # only contended during the v third
                        if dest is vT or tcn % 2 == 0:
                            ps = ps1.tile([128, 512], f32, tag="qkvps",
                                          name="qkv_ps")
                        else:
                            ps = pst.tile([128, 512], f32, tag="tp",
                                          name="qkv_ps2")
                        for ks in range(8):
                            nc.tensor.matmul(
                                ps, lhsT=wt[ks],
                                rhs=x_sb[ks][:, tcn * 512:(tcn + 1) * 512],
                                start=(ks == 0), stop=(ks == 7))
                        # evacuate with fused bias add: out = psum + b
                        nc.scalar.activation(
                            dest[:, tcn * 512:(tcn + 1) * 512], ps,
                            AF.Identity, bias=b_sb[:, ft:ft + 1], scale=1.0)
                        if dest is vT:
                            # v2 build interleaved: transpose the 4 ktiles of
                            # this freshly written v token-chunk
                            for kt in range(4 * tcn, 4 * tcn + 4):
                                pt = pst.tile([128, 128], f32r, tag="tp",
                                              name="tr_ps")
                                nc.tensor.transpose(
                                    pt, vT[:, kt * 128:(kt + 1) * 128], ident)
                                base = kt * V2W
                                nc.vector.tensor_copy(
                                    v2[:, base:base + 64], pt[:, 0:64])
                                nc.vector.tensor_copy(
                                    v2[:, base + 65:base + 129],
                                    pt[:, 64:128])

                # ---- attention for this pair, all 4 q-chunks ----
                for qc in range(4):
                    collq = coll_pool.tile([2, 512], f32, tag="cq",
                                           name="collq")
                    collrq = coll_pool.tile([2, 512], f32r, tag="cr",
                                            name="collrq")
                    ngr = 2 * (qc + 1)      # groups of 2 ktiles
                    c_ext = [cxps.tile([65, 512], f32, tag="cext",
                                       name="c_ext") for _ in range(2)]
                    for g in range(ngr):
                        diag = g >= ngr - 2
                        m = g - (ngr - 2)
                        scp = [scps.tile([128, 1024], f32, tag="sc",
                                         name="sc_ps") for _ in range(2)]
                        for j in range(2):
                            kt = 2 * g + j
                            roff = min((2 * m + j) * 128, 256) if diag else 0
                            for side in range(2):
                                poff = side * 64
                                nc.tensor.matmul(
                                    scp[side][:, j * 512 + roff:
                                              (j + 1) * 512],
                                    lhsT=kp[poff:poff + 64,
                                            kt * 128:(kt + 1) * 128],
                                    rhs=qp[poff:poff + 64,
                                           qc * 512 + roff:(qc + 1) * 512],
                                    start=True, stop=True)
                        ee = []
                        for side in range(2):
                            e = esb.tile([128, 1024], f32r, tag="e",
                                         name="e_sb")
                            if diag and m == 1:
                                # only ktiles r=2,3 live here; exp just the
                                # valid column ranges
                                nc.scalar.activation(
                                    e[:, 256:512], scp[side][:, 256:512],
                                    AF.Exp, scale=0.125)
                                nc.scalar.activation(
                                    e[:, 896:1024], scp[side][:, 896:1024],
                                    AF.Exp, scale=0.125)
                            else:
                                nc.scalar.activation(e, scp[side], AF.Exp,
                                                     scale=0.125)
                            if diag:
                                # in-tile causal boundary: 128-wide triangle
                                # per diagonal ktile
                                for j in range(2):
                                    r = 2 * m + j
                                    c0 = j * 512 + r * 128
                                    nc.vector.tensor_mul(
                                        e[:, c0:c0 + 128],
                                        e[:, c0:c0 + 128], tri_sb)
                            ee.append(e)
                        for j in range(2):
                            kt = 2 * g + j
                            r = 2 * m + j
                            roff = r * 128 if diag else 0
                            for side in range(2):
                                vb = kt * V2W + side * 65
                                nc.tensor.matmul(
                                    c_ext[side][:, roff:512],
                                    lhsT=v2[:, vb:vb + 65],
                                    rhs=ee[side][:, j * 512 + roff:
                                                 (j + 1) * 512],
                                    start=(g == 0 and j == 0),
                                    stop=(g == ngr - 1 and j == 1))
                    for side in range(2):
                        poff = side * 64
                        # engine APs need 32-aligned partition bases, so the
                        # denominator row (psum partition 64) is staged on
                        # partition 64 and moved to the collector row by DMA
                        dst = stg.tile([65, 512], f32, tag="dstage",
                                       name="dstage")
                        nc.vector.tensor_copy(dst[64:65, :],
                                              c_ext[side][64:65, :])
                        nc.sync.dma_start(out=collq[side:side + 1, :],
                                          in_=dst[64:65, :])
                        nc.vector.tensor_copy(
                            ctx_sb[pair][poff:poff + 64,
                                         qc * 512:(qc + 1) * 512],
                            c_ext[side][0:64, :])
                    # normalize: batched reciprocal of both heads' rows, then
                    # partition-broadcast each row by a step-0 DMA
                    with nc.allow_low_precision(reason="f32r == f32 storage"):
                        nc.vector.reciprocal(collrq, collq)
                    dsc = dscr.tile([2, 512], f32r, tag="ds", name="dsc")
                    nc.scalar.dma_start(out=dsc, in_=collrq)
                    # one [128,512] tile, each head's reciprocal row broadcast
                    # over its own partition half so the multiply's operand
                    # base partitions match
                    bcast = bcq.tile([128, 512], f32r, tag="bc", name="bcast")
                    for side in range(2):
                        nc.scalar.dma_start(
                            out=bcast[side * 64:(side + 1) * 64, :],
                            in_=dsc[side:side + 1, :].to_broadcast(
                                [64, 512]))
                    for side in range(2):
                        poff = side * 64
                        cslice = ctx_sb[pair][poff:poff + 64,
                                              qc * 512:(qc + 1) * 512]
                        nc.vector.tensor_mul(cslice, cslice,
                                             bcast[poff:poff + 64, :])

        # ---------------- tail: out projection ----------------
        with tc.tile_pool(name="wop", bufs=1) as wop, \
             tc.tile_pool(name="yps", bufs=4, space="PSUM") as yps, \
             tc.tile_pool(name="ysbp", bufs=4) as ysbp:
            w_o_sb = [wop.tile([128, C], f32r, tag=f"wo{i}", name=f"wo_sb{i}")
                      for i in range(4)]
            for f in range(4):
                nc.gpsimd.dma_start(out=w_o_sb[f],
                                    in_=w_o[f * 128:(f + 1) * 128, :])
            for tt in range(TK):
                for oc in range(2):
                    yp = yps.tile([128, 512], f32, tag="yp", name="y_ps")
                    for f in range(4):
                        nc.tensor.matmul(
                            yp, lhsT=ctx_sb[f][:, tt * 128:(tt + 1) * 128],
                            rhs=w_o_sb[f][:, oc * 512:(oc + 1) * 512],
                            start=(f == 0), stop=(f == 3))
                    ysb = ysbp.tile([128, 512], f32, tag="ysb", name="y_sb")
                    if oc == 0:
                        nc.scalar.activation(ysb, yp, AF.Copy)
                    else:
                        nc.vector.tensor_copy(ysb, yp)
                    nc.sync.dma_start(
                        out=y_d[tt * 128:(tt + 1) * 128,
                                oc * 512:(oc + 1) * 512],
                        in_=ysb)

    nc.compile()
    return nc


def _host_inputs(x, w_qkv, b_qkv, w_out):
    """Build the 8 per-core input maps."""
    tri = (np.arange(128)[:, None] <= np.arange(128)[None, :]).astype(
        np.float32)

    xt = [np.ascontiguousarray(x[b].T) for b in range(B)]      # [C, T] each
    in_maps = []
    for core in range(NCORES):
        b, hg = core // 2, core % 2
        cs = slice(hg * FQ, (hg + 1) * FQ)
        w_slice = np.concatenate(
            [w_qkv[:, cs], w_qkv[:, C + hg * FQ: C + (hg + 1) * FQ],
             w_qkv[:, 2 * C + hg * FQ: 2 * C + (hg + 1) * FQ]], axis=1)
        b_slice = np.concatenate(
            [b_qkv[cs], b_qkv[C + hg * FQ: C + (hg + 1) * FQ],
             b_qkv[2 * C + hg * FQ: 2 * C + (hg + 1) * FQ]])
        in_maps.append({
            "x_t": xt[b],
            "w_s": np.ascontiguousarray(w_slice),
            "b_s": np.ascontiguousarray(b_slice),
            "w_o": np.ascontiguousarray(w_out[hg * FQ:(hg + 1) * FQ, :]),
            "tri": tri,
            "idm": np.eye(128, dtype=np.float32),
            "onec": np.ones((128, 64), dtype=np.float32),
        })
    return in_maps


def get_program():
    if "nc" not in _CACHE:
        _CACHE["nc"] = _build_program()
    return _CACHE["nc"]


def kernel(x, w_qkv, b_qkv, w_out, b_out):
    from concourse.bass_utils import run_bass_kernel_spmd

    x = np.asarray(x, dtype=np.float32)
    w_qkv = np.asarray(w_qkv, dtype=np.float32)
    b_qkv = np.asarray(b_qkv, dtype=np.float32)
    w_out = np.asarray(w_out, dtype=np.float32)
    b_out = np.asarray(b_out, dtype=np.float32)

    nc = get_program()
    in_maps = _host_inputs(x, w_qkv, b_qkv, w_out)
    res = run_bass_kernel_spmd(nc, in_maps, core_ids=list(range(NCORES)))

    out = np.empty((B, T, C), dtype=np.float32)
    for b in range(B):
        out[b] = res.results[2 * b]["y"] + res.results[2 * b + 1]["y"] + b_out
    return out



# revision 48
# speedup vs baseline: 1.1572x; 1.1572x over previous
"""Causal self-attention (B=4, T=2048, C=1024, H=16, D=64) on 8 trn2 cores.

Sharding: core c handles batch b = c//2 and head-group hg = c%2 (8 heads).
The final 2-way partial-sum + bias happens on host.

Per-core program (v2 — fp8 DoubleRow rework):
  - qkv projections run as 3-term fp8e4m3 DoubleRow GEMMs:
    w64 = 64*w split into fp8 hi+lo, x split into fp8 hi+lo (residual
    unscaled — fp8 subnormals give a ~2^-10 absolute floor), and
    hi*hi + lo*hi + hi*lo accumulate in one PSUM at a common scale.
  - q,k stored as fp8 [64, 2, T] pair tiles (head at partition 0/32,
    feature halves in the 2-slot dim) so the 64-deep scores contraction
    runs as a single DoubleRow matmul per key-tile.
  - k bias is dropped (provably cancels in softmax); v bias is folded
    into b_out on host (softmax weights sum to 1); q bias applied at
    evacuation time.
  - e = exp(scores) in bf16 on ACT; attn@v and out_proj in bf16; the
    ones-column in v2 makes attn@v also emit the softmax denominator.
  - evacuations on Pool/DVE to keep ACT free for exp (the ACT floor).
"""

import os
import sys

for _p in ("/opt/trn_rl_repo", "/root/.axon_site/_ro/trn_rl_repo"):
    if os.path.isdir(_p) and _p not in sys.path:
        sys.path.insert(0, _p)

import numpy as np
import ml_dtypes

B, T, C = 4, 2048, 1024
H, D = 16, 64
NCORES = 8
HPC = 8          # heads per core
FQ = HPC * D     # 512 per-core q (=k=v) feature count
TK = T // 128    # 16 token tiles of 128
V2W = 132        # v2 per-ktile width: (64 v + 1 one + 1 pad) * 2 sides

F8 = ml_dtypes.float8_e4m3
BF = ml_dtypes.bfloat16

_CACHE = {}


def _build_program():
    import concourse.bacc as bacc
    import concourse.tile as tile
    import concourse.mybir as mybir
    from contextlib import ExitStack

    f32 = mybir.dt.float32
    bf16 = mybir.dt.bfloat16
    fp8 = mybir.dt.float8e4
    AF = mybir.ActivationFunctionType
    ALU = mybir.AluOpType
    DR = mybir.MatmulPerfMode.DoubleRow

    nc = bacc.Bacc("TRN2", target_bir_lowering=False, debug=False)

    xhi_d = nc.dram_tensor("x8hi", [C, T], fp8, kind="ExternalInput").ap()
    xlo_d = nc.dram_tensor("x8lo", [C, T], fp8, kind="ExternalInput").ap()
    wqkh_d = nc.dram_tensor("wqk8hi", [C, 1024], fp8, kind="ExternalInput").ap()
    wvh_d = nc.dram_tensor("wv8hi", [C, FQ], fp8, kind="ExternalInput").ap()
    wvl_d = nc.dram_tensor("wv8lo", [C, FQ], fp8, kind="ExternalInput").ap()
    wo_d = nc.dram_tensor("wo16", [FQ, C], bf16, kind="ExternalInput").ap()
    bq_d = nc.dram_tensor("bq64", [64, 8], f32, kind="ExternalInput").ap()
    tri_d = nc.dram_tensor("tri16", [128, 128], bf16, kind="ExternalInput").ap()
    y_d = nc.dram_tensor("y", [T, C], f32, kind="ExternalOutput").ap()
    dbg = os.environ.get("K_DEBUG", "0") == "1"
    if dbg:
        cpre_d = nc.dram_tensor("cpre", [128, 512], bf16,
                                kind="ExternalOutput").ap()
        rec_d = nc.dram_tensor("recd", [2, 512], bf16,
                               kind="ExternalOutput").ap()
        bcd_d = nc.dram_tensor("bcd", [128, 2 * 512], bf16,
                               kind="ExternalOutput").ap()
        edbg_d = nc.dram_tensor("edbg", [128, 1024], bf16,
                                kind="ExternalOutput").ap()
        qdbg_d = nc.dram_tensor("qdbg", [64, 2 * T], fp8,
                                kind="ExternalOutput").ap()
        kdbg_d = nc.dram_tensor("kdbg", [64, 2 * T], fp8,
                                kind="ExternalOutput").ap()
        vdbg_d = nc.dram_tensor("vdbg", [128, 4 * TK * V2W], bf16,
                                kind="ExternalOutput").ap()
        cdbg_d = nc.dram_tensor("cdbg", [128, T], bf16,
                                kind="ExternalOutput").ap()

    with tile.TileContext(nc) as tc, ExitStack() as ctx:
        pp = ctx.enter_context(tc.tile_pool(name="persist", bufs=1))
        # whole-kernel persistent SBUF tensors (2D tiles + reshaped views)
        x_hi = pp.tile([128, 8 * T], fp8, tag="xhi", name="x_hi")
        x_lo = pp.tile([128, 8 * T], fp8, tag="xlo", name="x_lo")
        wqk_hi = pp.tile([128, 8 * 1024], fp8, tag="wqh", name="wqk_hi")
        wv_hi = pp.tile([128, 8 * FQ], fp8, tag="wvh", name="wv_hi")
        wv_lo = pp.tile([128, 8 * FQ], fp8, tag="wvl", name="wv_lo")
        wo_sb = pp.tile([128, 4 * C], bf16, tag="wo", name="wo_sb")
        bq_sb = pp.tile([64, 8], f32, tag="bq", name="bq_sb")
        tri_sb = pp.tile([128, 128], bf16, tag="tri", name="tri_sb")
        qT = [pp.tile([64, 2 * T], fp8, tag=f"q{p}", name=f"qT{p}")
              for p in range(4)]
        kT = [pp.tile([64, 2 * T], fp8, tag=f"k{p}", name=f"kT{p}")
              for p in range(4)]
        v2all = pp.tile([128, 4 * TK * V2W], bf16, tag="v2", name="v2all")
        ctx4 = [pp.tile([128, T], bf16, tag=f"c{p}", name=f"ctx4_{p}")
                for p in range(4)]

        xhi_v = x_hi.rearrange("p (s t) -> p s t", s=8)
        xlo_v = x_lo.rearrange("p (s t) -> p s t", s=8)
        wqkh_v = wqk_hi.rearrange("p (s f) -> p s f", s=8)
        wvh_v = wv_hi.rearrange("p (s f) -> p s f", s=8)
        wvl_v = wv_lo.rearrange("p (s f) -> p s f", s=8)
        qT_v = [t.rearrange("p (s t) -> p s t", s=2) for t in qT]
        kT_v = [t.rearrange("p (s t) -> p s t", s=2) for t in kT]
        v2_v = v2all.rearrange("p (pr k sd w) -> p pr k sd w", pr=4, k=TK,
                               sd=2)

        # weight/misc loads: scalar queue (keeps SP free for x), split so the
        # first projection's weight slices land first
        nc.sync.dma_start(out=tri_sb, in_=tri_d)
        nc.sync.dma_start(out=bq_sb, in_=bq_d)
        wo_v = wo_sb.rearrange("p (f o) -> p f o", f=4)
        wqkh_r = wqkh_d.rearrange("(s p) f -> p s f", p=128)
        # pair-0 q/k weight slices first so the first matmul starts early
        for lo_, hi_ in ((0, 128), (512, 640)):     # q/k pair-0 columns
            nc.scalar.dma_start(out=wqkh_v[:, :, lo_:hi_],
                                in_=wqkh_r[:, :, lo_:hi_])
        nc.scalar.dma_start(out=wvh_v,
                            in_=wvh_d.rearrange("(s p) f -> p s f", p=128))
        nc.scalar.dma_start(out=wvl_v,
                            in_=wvl_d.rearrange("(s p) f -> p s f", p=128))
        for lo_, hi_ in ((128, 512), (640, 1024)):  # remaining q/k columns
            nc.scalar.dma_start(out=wqkh_v[:, :, lo_:hi_],
                                in_=wqkh_r[:, :, lo_:hi_])
        nc.scalar.dma_start(out=wo_v,
                            in_=wo_d.rearrange("(f p) o -> p f o", p=128))

        xhi_r = xhi_d.rearrange("(s p) t -> p s t", p=128)
        xlo_r = xlo_d.rearrange("(s p) t -> p s t", p=128)
        # x loads: one DMA per (tensor, slice-pair), full token range -- the
        # DRAM rows per partition stay contiguous (few descriptors) and
        # matmul term ck depends only on its slice-pair
        for ck in range(4):
            nc.sync.dma_start(out=xhi_v[:, 2 * ck:2 * ck + 2, :],
                              in_=xhi_r[:, 2 * ck:2 * ck + 2, :])
            nc.sync.dma_start(out=xlo_v[:, 2 * ck:2 * ck + 2, :],
                              in_=xlo_r[:, 2 * ck:2 * ck + 2, :])

        # ones columns of v2 (for the softmax denominator rows)
        nc.vector.memset(v2_v[:, :, :, 0:1, 64:65], 1.0)
        nc.vector.memset(v2_v[:, :, :, 1:2, 64:65], 1.0)
        # warm the ACT Exp table during the initial DMA wait
        warm = pp.tile([1, 16], bf16, tag="warm", name="warm")
        nc.scalar.activation(warm, tri_sb[0:1, 0:16], AF.Exp, scale=1.0)

        qkps = ctx.enter_context(
            tc.tile_pool(name="qkps", bufs=2, space="PSUM"))
        scps = ctx.enter_context(
            tc.tile_pool(name="scps", bufs=2, space="PSUM"))
        cxps = ctx.enter_context(
            tc.tile_pool(name="cxps", bufs=2, space="PSUM"))
        esb = ctx.enter_context(tc.tile_pool(name="esb", bufs=4))
        coll = ctx.enter_context(tc.tile_pool(name="coll", bufs=3))
        bcq = ctx.enter_context(tc.tile_pool(name="bcq", bufs=3))
        ystg = ctx.enter_context(tc.tile_pool(name="ystg", bufs=3))
        dscr = ctx.enter_context(
            tc.tile_pool(name="dscr", bufs=4, space="DRAM"))

        def rr_eng():
            return nc.gpsimd

        def terms():
            yield xhi_v, None, wvh_v
            yield xhi_v, None, wvl_v
            yield xlo_v, None, wvh_v

        def emit_qk(tau, pair, c, eng=None):
            """q or k projection for one pair's 2 heads, one token chunk.

            Two fp8 terms only (x_hi + x_lo, against w_hi): q,k are
            re-quantized to fp8 for the scores matmul anyway, so the
            dropped w_lo term is below that quantization floor.
            """
            wcol = (tau * 4 + pair) * 128
            ps = qkps.tile([128, 512], f32, tag="p512", name="qk_ps")
            ti = 0
            for xt in (xhi_v, xlo_v):
                for ck in range(4):
                    nc.tensor.matmul(
                        ps,
                        lhsT=wqkh_v[:, 2 * ck:2 * ck + 2, wcol:wcol + 128],
                        rhs=xt[:, 2 * ck:2 * ck + 2, c * 512:(c + 1) * 512],
                        start=(ti == 0), stop=(ti == 7), perf_mode=DR)
                    ti += 1
            dest = qT[pair] if tau == 0 else kT[pair]
            for s in range(2):
                dst = dest[:, s * T + c * 512: s * T + (c + 1) * 512]
                if tau == 0 and s == 0:
                    # q slot 0: psum/64 + bias -> fp8 on DVE
                    nc.vector.tensor_scalar(
                        out=dst, in0=ps[0:64, :],
                        scalar1=1.0 / 64.0,
                        scalar2=bq_sb[:, pair * 2:pair * 2 + 1],
                        op0=ALU.mult, op1=ALU.add)
                elif tau == 0:
                    # q slot 1 on ACT (parallel drain of the same psum)
                    nc.scalar.activation(
                        dst, ps[64:128, :], AF.Identity, scale=1.0 / 64.0,
                        bias=bq_sb[:, pair * 2 + 1:pair * 2 + 2])
                elif s == 0:
                    # k: bias provably cancels in softmax; scale only
                    nc.vector.tensor_scalar_mul(dst, ps[0:64, :], 1.0 / 64.0)
                else:
                    nc.scalar.activation(dst, ps[64:128, :], AF.Identity,
                                         scale=1.0 / 64.0)

        def emit_v(kt, half, eng=None):
            """v for 2 pairs' 4 heads, one key tile, [keys, feat] layout."""
            eng = eng or nc.vector
            ps = qkps.tile([128, 512], f32, tag="p512", name="v_ps")
            ti = 0
            for xt, _, wt in terms():
                for ck in range(4):
                    nc.tensor.matmul(
                        ps[:, 0:256],
                        lhsT=xt[:, 2 * ck:2 * ck + 2, kt * 128:(kt + 1) * 128],
                        rhs=wt[:, 2 * ck:2 * ck + 2,
                               half * 256:(half + 1) * 256],
                        start=(ti == 0), stop=(ti == 11), perf_mode=DR)
                    ti += 1
            psr = ps.rearrange("p (pr sd f) -> p pr sd f", pr=4, sd=2)
            eng.tensor_scalar_mul(
                v2_v[:, 2 * half:2 * half + 2, kt:kt + 1, :, 0:64],
                psr[:, 0:2, :, :], 1.0 / 64.0)

        # ---- filler machinery: interleave qkv/outproj/DMA work between ----
        # ---- attention groups at fine granularity so the in-order PE  ----
        # ---- sequencer never stalls on an exp-dependent ctx matmul    ----
        filler = []          # list of (deadline_idx, nb_groups, cost, fn)
        fill_state = {"emitted": 0.0, "groups": 0}
        FILL_PER_GROUP = float(os.environ.get("K_FPG", "450"))
        K_BOOST = float(os.environ.get("K_BOOST", "2600"))
        K_F0 = int(os.environ.get("K_F0", "4"))
        K_F123 = int(os.environ.get("K_F123", "1"))
        K_NB = int(os.environ.get("K_NB", "4"))

        def pump(force=0, boost=0.0):
            fill_state["groups"] += 1
            fill_state["bonus"] = fill_state.get("bonus", 0.0) + boost
            budget = (fill_state["groups"] * FILL_PER_GROUP
                      + fill_state["bonus"])
            popped = 0
            while filler:
                if popped < force:
                    pass
                elif (fill_state["emitted"] >= budget
                      or fill_state["groups"] < filler[0][1]):
                    break
                _, _, cost, fn = filler.pop(0)
                fn()
                fill_state["emitted"] += cost
                popped += 1

        def attn(pair, qc, force=0, boost=0.0):
            ngr = 2 * (qc + 1)
            # interleave the two heads' (sides') group chains: doubles the
            # scores->exp->ctx pipeline distance on the 2-tile psum rings
            ce = [cxps.tile([65, 512], f32, tag="cx", name="c_ext")
                  for _ in range(2)]
            for g in range(ngr):
                diag = g >= ngr - 2
                m = g - (ngr - 2)
                for side in range(2):
                    scp = scps.tile([128, 1024], f32, tag="sc", name="sc_ps")
                    for j in range(2):
                        kt = 2 * g + j
                        roff = min((2 * m + j) * 128, 256) if diag else 0
                        nc.tensor.matmul(
                            scp[:, j * 512 + roff:(j + 1) * 512],
                            lhsT=kT_v[pair][32 * side:32 * side + 32, :,
                                            kt * 128:(kt + 1) * 128],
                            rhs=qT_v[pair][32 * side:32 * side + 32, :,
                                           qc * 512 + roff:(qc + 1) * 512],
                            start=True, stop=True, perf_mode=DR)
                    e = esb.tile([128, 1024], bf16, tag="e", name="e_sb")
                    if diag and m == 1:
                        nc.scalar.activation(e[:, 256:512], scp[:, 256:512],
                                             AF.Exp, scale=0.125)
                        nc.scalar.activation(e[:, 896:1024], scp[:, 896:1024],
                                             AF.Exp, scale=0.125)
                    else:
                        nc.scalar.activation(e, scp, AF.Exp, scale=0.125)
                    if diag:
                        for j in range(2):
                            r = 2 * m + j
                            c0 = j * 512 + r * 128
                            nc.gpsimd.tensor_mul(
                                e[:, c0:c0 + 128], e[:, c0:c0 + 128], tri_sb)
                    # filler lands between the exp and the exp-dependent ctx
                    # matmuls, so the in-order PE sequencer keeps executing
                    pump(force=force if side == 0 else 0, boost=boost)
                    for j in range(2):
                        kt = 2 * g + j
                        r = 2 * m + j
                        roff = r * 128 if diag else 0
                        nc.tensor.matmul(
                            ce[side][:, roff:512],
                            lhsT=v2all[:, (pair * TK + kt) * V2W + side * 66:
                                       (pair * TK + kt) * V2W + side * 66 + 65],
                            rhs=e[:, j * 512 + roff:(j + 1) * 512],
                            start=(g == 0 and j == 0),
                            stop=(g == ngr - 1 and j == 1))
            for side in range(2):
                cs = ctx4[pair][64 * side:64 * side + 64,
                                qc * 512:(qc + 1) * 512]
                nc.vector.tensor_copy(cs, ce[side][0:64, :])
                rec = coll.tile([1, 512], bf16, tag="rc", name="recip")
                with nc.allow_low_precision(reason="softmax recip in bf16"):
                    nc.vector.reciprocal(rec, ce[side][64:65, :])
                bc = bcq.tile([128, 512], bf16, tag="bc", name="bcast")
                nc.gpsimd.partition_broadcast(bc, rec)
                if dbg and pair == 0 and qc == 0:
                    nc.sync.dma_start(
                        out=cpre_d[64 * side:64 * side + 64, :],
                        in_=cs)
                    nc.sync.dma_start(out=rec_d[side:side + 1, :], in_=rec)
                    nc.sync.dma_start(
                        out=bcd_d[:, side * 512:(side + 1) * 512],
                        in_=bc)
                nc.vector.tensor_mul(cs, cs, bc[64 * side:64 * side + 64, :])

        def outproj(tt, oc):
            yp = qkps.tile([128, 512], f32, tag="p512", name="y_ps")
            for f in range(4):
                nc.tensor.matmul(
                    yp, lhsT=ctx4[f][:, tt * 128:(tt + 1) * 128],
                    rhs=wo_sb[:, f * C + oc * 512: f * C + (oc + 1) * 512],
                    start=(f == 0), stop=(f == 3))
            ys = ystg.tile([128, 512], f32, tag="ys", name="y_sb")
            nc.vector.tensor_copy(ys, yp)
            nc.sync.dma_start(
                out=y_d[tt * 128:(tt + 1) * 128, oc * 512:(oc + 1) * 512],
                in_=ys)

        # ---- emission schedule (program order = scheduler priority) ----
        SEQ = [(p, qc) for p in range(4) for qc in range(4)]
        IDX = {pq: i for i, pq in enumerate(SEQ)}

        def unit(dl, cost, fn, nb=0):
            filler.append((dl, nb, cost, fn))

        def flush_until(idx):
            while filler and filler[0][0] <= idx:
                _, _, cost, fn = filler.pop(0)
                fn()
                fill_state["emitted"] += cost

        emit_qk(0, 0, 0, eng=nc.vector)
        emit_qk(1, 0, 0, eng=nc.vector)

        def qk_unit(tau, pair, c):
            unit(IDX[(pair, c)], 854.0,
                 lambda t=tau, p=pair, cc=c: emit_qk(t, p, cc))

        def v_unit(kt, half):
            # half 0 feeds pairs 0-1, half 1 feeds pairs 2-3
            dl = IDX[(0 if half == 0 else 2, kt // 4)]
            unit(max(dl, 1), 640.0, lambda k=kt, h=half: emit_v(k, h))

        for kt in range(4):
            v_unit(kt, 0)
        qk_unit(0, 0, 1)
        qk_unit(1, 0, 1)
        for kt in range(4, 8):
            v_unit(kt, 0)
        for c in range(2, 4):
            qk_unit(0, 0, c)
            qk_unit(1, 0, c)
            for kt in range(4 * c, 4 * c + 4):
                v_unit(kt, 0)
        for pair in range(1, 4):
            for c in range(4):
                qk_unit(0, pair, c)
                qk_unit(1, pair, c)
                if pair == 2:
                    for kt in range(4 * c, 4 * c + 4):
                        v_unit(kt, 1)

        for i, (pair, qc) in enumerate(SEQ):
            flush_until(i)
            attn(pair, qc,
                 force=(K_F0 if i == 0 else (K_F123 if i <= 3 else 0)),
                 boost=K_BOOST if pair == 3 else 0.0)
            if pair == 3:
                for tt in range(4 * qc, 4 * qc + 4):
                    for oc in range(2):
                        unit(99, 853.0, lambda t=tt, o=oc: outproj(t, o),
                             nb=fill_state["groups"] + K_NB)
        while filler:
            _, _, cost, fn = filler.pop(0)
            fn()
        if dbg:
            nc.sync.dma_start(out=qdbg_d, in_=qT[0])
            nc.sync.dma_start(out=kdbg_d, in_=kT[0])
            nc.sync.dma_start(out=vdbg_d, in_=v2all)
            nc.sync.dma_start(out=cdbg_d, in_=ctx4[0])

    nc.compile()
    return nc


def _host_inputs(x, w_qkv, b_qkv, w_out):
    """Build the 8 per-core input maps."""
    f32 = np.float32
    tri = (np.arange(128)[:, None] <= np.arange(128)[None, :]).astype(BF)

    def split8(a):
        hi = a.astype(F8)
        lo = (a - hi.astype(f32)).astype(F8)
        return hi, lo

    xs = [split8(np.ascontiguousarray(x[b].T)) for b in range(B)]

    # per-partition index maps for the qk weight arrangement
    p = np.arange(128)
    head_of_p = (p % 64) // 32          # head within pair
    feat_of_p = 32 * (p // 64) + (p % 32)

    in_maps = []
    for core in range(NCORES):
        b, hg = core // 2, core % 2
        cols = np.empty(1024, dtype=np.int64)
        for tau in range(2):
            for pair in range(4):
                base = (tau * 4 + pair) * 128
                cols[base:base + 128] = (tau * C + hg * FQ
                                         + (pair * 2 + head_of_p) * 64
                                         + feat_of_p)
        wqk = np.ascontiguousarray(w_qkv[:, cols]) * 64.0
        wqk_hi, wqk_lo = split8(wqk)
        wv = w_qkv[:, 2 * C + hg * FQ: 2 * C + (hg + 1) * FQ] * 64.0
        wv_hi, wv_lo = split8(np.ascontiguousarray(wv))

        # wo rows ordered as ctx4 partitions: (pair, side*64 + d)
        po = np.arange(128)
        rows = np.empty(FQ, dtype=np.int64)
        for f in range(4):
            rows[f * 128:(f + 1) * 128] = (hg * FQ + (2 * f + po // 64) * 64
                                           + po % 64)
        wo16 = np.ascontiguousarray(w_out[rows, :]).astype(BF)

        # q bias, 64*b, laid out [64, (pair, slot)]
        p64 = np.arange(64)
        bq64 = np.empty((64, 8), dtype=f32)
        for pair in range(4):
            for s in range(2):
                idx = hg * FQ + (pair * 2 + p64 // 32) * 64 + 32 * s + p64 % 32
                bq64[:, pair * 2 + s] = b_qkv[idx]
        in_maps.append({
            "x8hi": xs[b][0], "x8lo": xs[b][1],
            "wqk8hi": wqk_hi, "wqk8lo": wqk_lo,
            "wv8hi": wv_hi, "wv8lo": wv_lo,
            "wo16": wo16, "bq64": bq64, "tri16": tri,
        })
    return in_maps


def get_program():
    if "nc" not in _CACHE:
        _CACHE["nc"] = _build_program()
    return _CACHE["nc"]


def kernel(x, w_qkv, b_qkv, w_out, b_out):
    from concourse.bass_utils import run_bass_kernel_spmd

    x = np.asarray(x, dtype=np.float32)
    w_qkv = np.asarray(w_qkv, dtype=np.float32)
    b_qkv = np.asarray(b_qkv, dtype=np.float32)
    w_out = np.asarray(w_out, dtype=np.float32)
    b_out = np.asarray(b_out, dtype=np.float32)

    nc = get_program()
    in_maps = _host_inputs(x, w_qkv, b_qkv, w_out)
    res = run_bass_kernel_spmd(nc, in_maps, core_ids=list(range(NCORES)))

    bias = b_out + b_qkv[2 * C:] @ w_out        # folded v-bias
    out = np.empty((B, T, C), dtype=np.float32)
    for b in range(B):
        out[b] = res.results[2 * b]["y"] + res.results[2 * b + 1]["y"] + bias
    return out
